# revision 9
# baseline (speedup 1.0000x reference)
"""BinaryLinear Trainium2 kernel.

Computes: out = binarize(x) @ binarize(weight - threshold).T * 2^round(clip(shift, -8, 0))

where binarize(v) = +1 if v >= 0 else -1, over x [B,S,IN], weight [OUT,IN].

Strategy (8 NeuronCores, tensor-parallel over OUT):
  - each core gets the full x and a 2048-row slice of weight/threshold
  - binarize to +/-0.5 (exact in bf16/fp8) with one fused DVE op; the
    missing x4 is folded into the final output scale
  - hardware DMA-transpose (bf16, xbar) produces the [contraction-
    partition] layout both matmul operands need, off the PE critical path
  - fp8 matmuls (values +/-0.5 exact in e4m3) accumulate into fp32 PSUM;
    optional DoubleRow perf mode contracts 256 rows/matmul for 2x PE rate
  - PSUM -> SBUF copy applies the power-of-two scale; result is bit-exact
"""

import sys

if "/opt/trn_rl_repo" not in sys.path:
    sys.path.insert(0, "/opt/trn_rl_repo")

import numpy as np

B, S, IN, OUT = 4, 2048, 4096, 16384
N_CORES = 8
O_SHARD = OUT // N_CORES  # 2048
P = 128  # partitions
N_CH = 512  # psum free-dim chunk (one bank of fp32)

USE_DOUBLE_ROW = True

# dev knobs (test.py only; harness uses defaults)
_TRACE = False
_LAST_RESULTS = None


def build_program(s_rows=B * S, o_shard=O_SHARD, kdim=IN, scale=1.0,
                  use_dr=USE_DOUBLE_ROW):
    """Trace the single-core SPMD program. Inputs: x [s_rows,kdim] f32,
    w [o_shard,kdim] f32, thr [o_shard,1] f32. Output: out [s_rows,o_shard] f32."""
    import concourse.bass as bass
    import concourse.mybir as mybir
    import concourse.tile as tile
    from concourse import bacc
    from concourse.alu_op_type import AluOpType

    f32 = mybir.dt.float32
    bf16 = mybir.dt.bfloat16
    fp8 = mybir.dt.float8e4

    n_sb = s_rows // P
    n_kt = kdim // P
    n_ob = o_shard // P
    n_oc = o_shard // N_CH

    nc = bacc.Bacc(None, target_bir_lowering=False, debug=False)

    x_d = nc.dram_tensor("x", [s_rows, kdim], f32, kind="ExternalInput")
    w_d = nc.dram_tensor("w", [o_shard, kdim], f32, kind="ExternalInput")
    t_d = nc.dram_tensor("thr", [o_shard, 1], f32, kind="ExternalInput")
    o_d = nc.dram_tensor("out", [s_rows, o_shard], f32, kind="ExternalOutput")

    with tile.TileContext(nc) as tc:
        with (
            tc.tile_pool(name="raw", bufs=3) as raw_pool,
            tc.tile_pool(name="b16", bufs=2) as b16_pool,
            tc.tile_pool(name="t16", bufs=3) as t16_pool,
            tc.tile_pool(name="w8", bufs=1) as w8_pool,
            tc.tile_pool(name="x8", bufs=3) as x8_pool,
            tc.tile_pool(name="outp", bufs=2) as out_pool,
            tc.tile_pool(name="thrp", bufs=2) as thr_pool,
            tc.tile_pool(name="ps", bufs=2, space="PSUM") as ps_pool,
        ):
            # --- weight prep: binarize + transpose into fp8 slab [p, kt, o] ---
            # w/thr loads go on gpsimd (SWDGE) so they don't queue ahead of
            # the x loads on the sync HWDGE ring; transposes ride the scalar
            # HWDGE ring.
            wslab = w8_pool.tile([P, n_kt, o_shard], fp8)
            for ob in range(n_ob):
                w_raw = raw_pool.tile([P, kdim], f32, name="w_raw", tag="raw")
                nc.gpsimd.dma_start(w_raw[:], w_d[ob * P:(ob + 1) * P, :])
                thr_t = thr_pool.tile([P, 1], f32, name="thr_t", tag="thr")
                nc.gpsimd.dma_start(thr_t[:], t_d[ob * P:(ob + 1) * P, :])
                wb16 = b16_pool.tile([P, kdim], bf16, name="wb16", tag="b16")
                # (w >= thr) - 0.5  ->  +/-0.5, exact
                nc.vector.tensor_scalar(
                    wb16[:], w_raw[:], thr_t[:], 0.5,
                    AluOpType.is_ge, AluOpType.subtract)
                wt16 = t16_pool.tile([P, n_kt, P], bf16, name="wt16", tag="t16")
                nc.scalar.dma_start(wt16[:], wb16[:], transpose=True)
                nc.vector.tensor_copy(wslab[:, :, ob * P:(ob + 1) * P], wt16[:])

            # --- main loop over s-blocks ---
            for sb in range(n_sb):
                x_raw = raw_pool.tile([P, kdim], f32, name="x_raw", tag="raw")
                nc.sync.dma_start(x_raw[:], x_d[sb * P:(sb + 1) * P, :])
                xb16 = b16_pool.tile([P, kdim], bf16, name="xb16", tag="b16")
                nc.vector.tensor_scalar(
                    xb16[:], x_raw[:], 0.0, 0.5,
                    AluOpType.is_ge, AluOpType.subtract)
                xt16 = t16_pool.tile([P, n_kt, P], bf16, name="xt16", tag="t16")
                nc.scalar.dma_start(xt16[:], xb16[:], transpose=True)
                x8 = x8_pool.tile([P, n_kt, P], fp8, name="x8", tag="x8")
                nc.vector.tensor_copy(x8[:], xt16[:])

                out_sb = out_pool.tile([P, o_shard], f32, name="out_sb", tag="out")
                pss = [
                    ps_pool.tile([P, N_CH], f32, name=f"ps{oc}", tag=f"ps{oc}")
                    for oc in range(n_oc)
                ]
                if use_dr:
                    assert n_kt % 2 == 0
                    for g in range(n_kt // 2):
                        for oc in range(n_oc):
                            nc.tensor.matmul(
                                pss[oc][:],
                                x8[:, 2 * g:2 * g + 2, :],
                                wslab[:, 2 * g:2 * g + 2, oc * N_CH:(oc + 1) * N_CH],
                                start=(g == 0), stop=(g == n_kt // 2 - 1),
                                perf_mode=mybir.MatmulPerfMode.DoubleRow)
                else:
                    for kt in range(n_kt):
                        for oc in range(n_oc):
                            nc.tensor.matmul(
                                pss[oc][:],
                                x8[:, kt, :],
                                wslab[:, kt, oc * N_CH:(oc + 1) * N_CH],
                                start=(kt == 0), stop=(kt == n_kt - 1))
                for oc in range(n_oc):
                    nc.scalar.activation(
                        out_sb[:, oc * N_CH:(oc + 1) * N_CH], pss[oc][:],
                        mybir.ActivationFunctionType.Copy,
                        bias=0.0, scale=float(scale))
                nc.gpsimd.dma_start(o_d[sb * P:(sb + 1) * P, :], out_sb[:])

    nc.compile()
    return nc


def _host_scale(shift_param):
    # 4x undoes the two 0.5 factors from binarizing to +/-0.5;
    # np.round is round-half-to-even, matching jnp.round.
    s = np.clip(np.float64(np.float32(shift_param)), -8.0, 0.0)
    return 4.0 * float(np.exp2(np.round(s)))


def kernel(x, weight, threshold, shift_param):
    from concourse.bass_utils import run_bass_kernel_spmd

    scale = _host_scale(shift_param)
    nc = build_program(scale=scale)

    xf = np.ascontiguousarray(x.astype(np.float32).reshape(B * S, IN))
    in_maps = []
    for c in range(N_CORES):
        sl = slice(c * O_SHARD, (c + 1) * O_SHARD)
        in_maps.append({
            "x": xf,
            "w": np.ascontiguousarray(weight[sl].astype(np.float32)),
            "thr": np.ascontiguousarray(
                threshold[sl].astype(np.float32).reshape(O_SHARD, 1)),
        })

    res = run_bass_kernel_spmd(nc, in_maps, list(range(N_CORES)), trace=_TRACE)
    global _LAST_RESULTS
    _LAST_RESULTS = res
    shards = [res.results[c]["out"] for c in range(N_CORES)]
    full = np.concatenate(shards, axis=1).reshape(B, S, OUT)
    return np.ascontiguousarray(full.astype(np.float32))


# revision 13
# speedup vs baseline: 1.0925x; 1.0925x over previous
"""BinaryLinear Trainium2 kernel.

Computes: out = binarize(x) @ binarize(weight - threshold).T * 2^round(clip(shift, -8, 0))

where binarize(v) = +1 if v >= 0 else -1, over x [B,S,IN], weight [OUT,IN].

Strategy (8 NeuronCores, tensor-parallel over OUT):
  - each core gets the full x and a 2048-row slice of weight/threshold
  - binarize to +/-0.5 (one fused DVE op, exact in fp8e4m3); the missing
    x4 is folded into the final output scale
  - fp8 DoubleRow matmuls (256 contraction rows per matmul, 2x PE rate)
    accumulate into fp32 PSUM; weights are the stationary operand (its
    DoubleRow pair-dim must be 16B-aligned -> grouped k-tile layout),
    x is the moving operand (pairs may be byte-adjacent -> packed layout)
  - the [contraction, .] layouts come from the hardware DMA-transpose
    (2-byte granularity): x is binarized straight to fp8 and transposed
    as packed fp8 pairs inside bf16-typed elements (half the bytes, no
    cast pass); w goes through a one-time bf16 value transpose + fp8 cast
  - the grouped-vs-packed pair mapping is reconciled by a one-time host
    interleave of weight columns; the [OUT, S] device output is
    transposed back on the host during the gather
  - result is bit-exact (all products +/-0.25, exact fp32 accumulation)
"""

import sys

if "/opt/trn_rl_repo" not in sys.path:
    sys.path.insert(0, "/opt/trn_rl_repo")

import numpy as np

B, S, IN, OUT = 4, 2048, 4096, 16384
N_CORES = 8
O_SHARD = OUT // N_CORES  # 2048
P = 128  # partitions
N_CH = 512  # psum free-dim chunk (one bank of fp32)

# dev knobs (test.py only; harness uses defaults)
_TRACE = False
_LAST_RESULTS = None


def build_program(s_rows=B * S, o_shard=O_SHARD, kdim=IN, scale=1.0):
    """Trace the single-core SPMD program.

    Inputs: x [s_rows,kdim] f32, w [o_shard,kdim] f32 (host-interleaved
    columns), thr [o_shard,1] f32. Output: outT [o_shard,s_rows] f32.
    """
    import concourse.bass as bass
    import concourse.mybir as mybir
    import concourse.tile as tile
    from concourse import bacc
    from concourse.alu_op_type import AluOpType

    f32 = mybir.dt.float32
    bf16 = mybir.dt.bfloat16
    fp8 = mybir.dt.float8e4

    n_g = kdim // 256      # DoubleRow groups (256 contraction rows each)
    n_kt = kdim // P       # 128-row k-tiles in the stationary slab
    n_ob = o_shard // P    # o-blocks of 128
    n_pass = n_ob // 4     # 4 o-blocks (psum banks) per pass
    n_sc = s_rows // N_CH  # s-chunks of 512
    assert s_rows % N_CH == 0 and o_shard % (4 * P) == 0 and kdim % 256 == 0

    nc = bacc.Bacc(None, target_bir_lowering=False, debug=False)

    x_d = nc.dram_tensor("x", [s_rows, kdim], f32, kind="ExternalInput")
    w_d = nc.dram_tensor("w", [o_shard, kdim], f32, kind="ExternalInput")
    t_d = nc.dram_tensor("thr", [o_shard, 1], f32, kind="ExternalInput")
    o_d = nc.dram_tensor("outT", [o_shard, s_rows], f32, kind="ExternalOutput")

    with tile.TileContext(nc) as tc:
        with (
            tc.tile_pool(name="raw", bufs=2) as raw_pool,
            tc.tile_pool(name="b8", bufs=3) as b8_pool,
            tc.tile_pool(name="wprep", bufs=1) as wprep_pool,
            tc.tile_pool(name="xt", bufs=3) as xt_pool,
            tc.tile_pool(name="w8", bufs=1) as w8_pool,
            tc.tile_pool(name="outp", bufs=4) as out_pool,
            tc.tile_pool(name="thrp", bufs=2) as thr_pool,
            tc.tile_pool(name="ps", bufs=2, space="PSUM") as ps_pool,
        ):
            # --- weight prep: per-pass stationary fp8 slabs [p, kt, 512] ---
            # slab[p, m, oq] = bin(w[ob*128+oq, 128*m+p]); with the host
            # column interleave this equals wb_logical at i = 256g+128j+p
            # for m = 2g+j, matching the packed x pairs below.
            wslabs = [
                w8_pool.tile([P, n_kt, 4 * P], fp8, name=f"wslab{ps}",
                             tag=f"wslab{ps}")
                for ps in range(n_pass)
            ]
            for ob in range(n_ob):
                w_raw = raw_pool.tile([P, kdim], f32, name="w_raw", tag="raw")
                nc.gpsimd.dma_start(w_raw[:], w_d[ob * P:(ob + 1) * P, :])
                thr_t = thr_pool.tile([P, 1], f32, name="thr_t", tag="thr")
                nc.gpsimd.dma_start(thr_t[:], t_d[ob * P:(ob + 1) * P, :])
                wb16 = wprep_pool.tile([P, kdim], bf16, name="wb16", tag="wb16")
                # (w >= thr) - 0.5  ->  +/-0.5, exact
                nc.vector.tensor_scalar(
                    wb16[:], w_raw[:], thr_t[:], 0.5,
                    AluOpType.is_ge, AluOpType.subtract)
                wt16 = wprep_pool.tile([P, n_kt, P], bf16, name="wt16",
                                       tag="wt16")
                nc.scalar.dma_start(wt16[:], wb16[:], transpose=True)
                nc.vector.tensor_copy(
                    wslabs[ob // 4][:, :, (ob % 4) * P:(ob % 4 + 1) * P],
                    wt16[:])

            # --- main loop over s-chunks of 512 ---
            for sc in range(n_sc):
                # x moving tile [p, g, 512 s] as packed fp8 pairs in bf16:
                # filled by 4 DMA-transposes (one per 128-row s-subblock)
                xtile = xt_pool.tile([P, n_g, N_CH], bf16, name="xtile",
                                     tag="xt")
                for sub in range(4):
                    s0 = sc * N_CH + sub * P
                    x_raw = raw_pool.tile([P, kdim], f32, name="x_raw",
                                          tag="raw")
                    nc.sync.dma_start(x_raw[:], x_d[s0:s0 + P, :])
                    xb8 = b8_pool.tile([P, kdim], fp8, name="xb8", tag="b8")
                    nc.vector.tensor_scalar(
                        xb8[:], x_raw[:], 0.0, 0.5,
                        AluOpType.is_ge, AluOpType.subtract)
                    nc.sync.dma_start(
                        xtile[:, :, sub * P:(sub + 1) * P],
                        xb8[:].bitcast(bf16), transpose=True)
                xt8 = xtile.bitcast(fp8)  # [p, g, 1024] (s,j interleaved)

                for ps in range(n_pass):
                    pss = [
                        ps_pool.tile([P, N_CH], f32, name=f"ps{i}",
                                     tag=f"ps{i}")
                        for i in range(4)
                    ]
                    for g in range(n_g):
                        rhs = xt8[:, g, :].rearrange("p (s j) -> p j s", j=2)
                        for i in range(4):
                            nc.tensor.matmul(
                                pss[i][:],
                                wslabs[ps][:, 2 * g:2 * g + 2,
                                           i * P:(i + 1) * P],
                                rhs,
                                start=(g == 0), stop=(g == n_g - 1),
                                perf_mode=mybir.MatmulPerfMode.DoubleRow)
                    for i in range(4):
                        ob = ps * 4 + i
                        ot = out_pool.tile([P, N_CH], f32, name="ot", tag="ot")
                        nc.scalar.activation(
                            ot[:], pss[i][:],
                            mybir.ActivationFunctionType.Copy,
                            bias=0.0, scale=float(scale))
                        nc.gpsimd.dma_start(
                            o_d[ob * P:(ob + 1) * P,
                                sc * N_CH:(sc + 1) * N_CH], ot[:])

    nc.compile()
    return nc


def _host_scale(shift_param):
    # 4x undoes the two 0.5 factors from binarizing to +/-0.5;
    # np.round is round-half-to-even, matching jnp.round.
    s = np.clip(np.float64(np.float32(shift_param)), -8.0, 0.0)
    return 4.0 * float(np.exp2(np.round(s)))


def _interleave_w_cols(w):
    """Host permutation so the device's grouped stationary layout pairs the
    same contraction rows as the packed moving layout: new col 256g+128j+p
    holds old col 256g+2p+j."""
    o, k = w.shape
    return np.ascontiguousarray(
        w.reshape(o, k // 256, 128, 2).transpose(0, 1, 3, 2).reshape(o, k))


def kernel(x, weight, threshold, shift_param):
    from concourse.bass_utils import run_bass_kernel_spmd

    scale = _host_scale(shift_param)
    nc = build_program(scale=scale)

    xf = np.ascontiguousarray(x.astype(np.float32).reshape(B * S, IN))
    wp = _interleave_w_cols(weight.astype(np.float32))
    in_maps = []
    for c in range(N_CORES):
        sl = slice(c * O_SHARD, (c + 1) * O_SHARD)
        in_maps.append({
            "x": xf,
            "w": np.ascontiguousarray(wp[sl]),
            "thr": np.ascontiguousarray(
                threshold[sl].astype(np.float32).reshape(O_SHARD, 1)),
        })

    res = run_bass_kernel_spmd(nc, in_maps, list(range(N_CORES)), trace=_TRACE)
    global _LAST_RESULTS
    _LAST_RESULTS = res
    shards = [res.results[c]["outT"] for c in range(N_CORES)]
    full_t = np.concatenate(shards, axis=0)  # [OUT, B*S]
    full = np.ascontiguousarray(full_t.T).reshape(B, S, OUT)
    return full.astype(np.float32)


# revision 16
# speedup vs baseline: 1.1214x; 1.0264x over previous
"""BinaryLinear Trainium2 kernel.

Computes: out = binarize(x) @ binarize(weight - threshold).T * 2^round(clip(shift, -8, 0))

where binarize(v) = +1 if v >= 0 else -1, over x [B,S,IN], weight [OUT,IN].

Strategy (8 NeuronCores, tensor-parallel over OUT):
  - each core gets the full x and a 2048-row slice of weight/threshold
  - binarize to +/-0.5 (one fused DVE op, exact in fp8e4m3); the missing
    x4 is folded into the final output scale
  - fp8 DoubleRow matmuls (256 contraction rows per matmul, 2x PE rate)
    accumulate into fp32 PSUM; weights are the stationary operand (its
    DoubleRow pair-dim must be 16B-aligned -> grouped k-tile layout),
    x is the moving operand (pairs may be byte-adjacent -> packed layout)
  - the [contraction, .] layouts come from the hardware DMA-transpose
    (2-byte granularity): x is binarized straight to fp8 and transposed
    as packed fp8 pairs inside bf16-typed elements (half the bytes, no
    cast pass); w goes through a one-time bf16 value transpose + fp8 cast
  - the grouped-vs-packed pair mapping is reconciled by a one-time host
    interleave of weight columns; the [OUT, S] device output is
    transposed back on the host during the gather
  - result is bit-exact (all products +/-0.25, exact fp32 accumulation)
"""

import sys

if "/opt/trn_rl_repo" not in sys.path:
    sys.path.insert(0, "/opt/trn_rl_repo")

import numpy as np

B, S, IN, OUT = 4, 2048, 4096, 16384
N_CORES = 8
O_SHARD = OUT // N_CORES  # 2048
P = 128  # partitions
N_CH = 512  # psum free-dim chunk (one bank of fp32)

# dev knobs (test.py only; harness uses defaults)
_TRACE = False
_LAST_RESULTS = None


def build_program(s_rows=B * S, o_shard=O_SHARD, kdim=IN, scale=1.0):
    """Trace the single-core SPMD program.

    Inputs: x [s_rows,kdim] f32, w [o_shard,kdim] f32 (host-interleaved
    columns), thr [o_shard,1] f32. Output: outT [o_shard,s_rows] f32.
    """
    import concourse.bass as bass
    import concourse.mybir as mybir
    import concourse.tile as tile
    from concourse import bacc
    from concourse.alu_op_type import AluOpType

    f32 = mybir.dt.float32
    bf16 = mybir.dt.bfloat16
    fp8 = mybir.dt.float8e4

    n_g = kdim // 256      # DoubleRow groups (256 contraction rows each)
    n_kt = kdim // P       # 128-row k-tiles in the stationary slab
    n_ob = o_shard // P    # o-blocks of 128
    n_pass = n_ob // 4     # 4 o-blocks (psum banks) per pass
    n_sc = s_rows // N_CH  # s-chunks of 512
    assert s_rows % N_CH == 0 and o_shard % (4 * P) == 0 and kdim % 256 == 0

    nc = bacc.Bacc(None, target_bir_lowering=False, debug=False)

    x_d = nc.dram_tensor("x", [s_rows, kdim], f32, kind="ExternalInput")
    w_d = nc.dram_tensor("w", [o_shard, kdim], f32, kind="ExternalInput")
    t_d = nc.dram_tensor("thr", [o_shard, 1], f32, kind="ExternalInput")
    o_d = nc.dram_tensor("outT", [o_shard, s_rows], f32, kind="ExternalOutput")

    with tile.TileContext(nc) as tc:
        with (
            tc.tile_pool(name="raw", bufs=2) as raw_pool,
            tc.tile_pool(name="b8", bufs=5) as b8_pool,
            tc.tile_pool(name="wprep", bufs=2) as wprep_pool,
            tc.tile_pool(name="xt", bufs=2) as xt_pool,
            tc.tile_pool(name="w8", bufs=1) as w8_pool,
            tc.tile_pool(name="outp", bufs=4) as out_pool,
            tc.tile_pool(name="thrp", bufs=2) as thr_pool,
            tc.tile_pool(name="ps", bufs=2, space="PSUM") as ps_pool,
        ):
            # --- weight prep: per-pass stationary fp8 slabs [p, kt, 512] ---
            # slab[p, m, oq] = bin(w[ob*128+oq, 128*m+p]); with the host
            # column interleave this equals wb_logical at i = 256g+128j+p
            # for m = 2g+j, matching the packed x pairs below.
            wslabs = [
                w8_pool.tile([P, n_kt, 4 * P], fp8, name=f"wslab{ps}",
                             tag=f"wslab{ps}")
                for ps in range(n_pass)
            ]
            for ob in range(n_ob):
                w_raw = raw_pool.tile([P, kdim], f32, name="w_raw", tag="raw")
                nc.gpsimd.dma_start(w_raw[:], w_d[ob * P:(ob + 1) * P, :])
                thr_t = thr_pool.tile([P, 1], f32, name="thr_t", tag="thr")
                nc.gpsimd.dma_start(thr_t[:], t_d[ob * P:(ob + 1) * P, :])
                wb16 = wprep_pool.tile([P, kdim], bf16, name="wb16", tag="wb16")
                # (w >= thr) - 0.5  ->  +/-0.5, exact
                nc.vector.tensor_scalar(
                    wb16[:], w_raw[:], thr_t[:], 0.5,
                    AluOpType.is_ge, AluOpType.subtract)
                wt16 = wprep_pool.tile([P, n_kt, P], bf16, name="wt16",
                                       tag="wt16")
                nc.scalar.dma_start(wt16[:], wb16[:], transpose=True)
                nc.vector.tensor_copy(
                    wslabs[ob // 4][:, :, (ob % 4) * P:(ob % 4 + 1) * P],
                    wt16[:])

            # --- main loop over s-chunks of 512 ---
            for sc in range(n_sc):
                # x moving tile [p, g, 512 s] as packed fp8 pairs in bf16:
                # filled by 4 DMA-transposes (one per 128-row s-subblock)
                xtile = xt_pool.tile([P, n_g, N_CH], bf16, name="xtile",
                                     tag="xt")
                xb8s = []
                for sub in range(4):
                    s0 = sc * N_CH + sub * P
                    x_raw = raw_pool.tile([P, kdim], f32, name="x_raw",
                                          tag="raw")
                    nc.sync.dma_start(x_raw[:], x_d[s0:s0 + P, :])
                    xb8 = b8_pool.tile([P, kdim], fp8, name="xb8", tag="b8")
                    nc.vector.tensor_scalar(
                        xb8[:], x_raw[:], 0.0, 0.5,
                        AluOpType.is_ge, AluOpType.subtract)
                    xb8s.append(xb8)
                for sub in range(4):
                    nc.scalar.dma_start(
                        xtile[:, :, sub * P:(sub + 1) * P],
                        xb8s[sub][:].bitcast(bf16), transpose=True)
                xt8 = xtile.bitcast(fp8)  # [p, g, 1024] (s,j interleaved)

                for ps in range(n_pass):
                    pss = [
                        ps_pool.tile([P, N_CH], f32, name=f"ps{i}",
                                     tag=f"ps{i}")
                        for i in range(4)
                    ]
                    for g in range(n_g):
                        rhs = xt8[:, g, :].rearrange("p (s j) -> p j s", j=2)
                        for i in range(4):
                            nc.tensor.matmul(
                                pss[i][:],
                                wslabs[ps][:, 2 * g:2 * g + 2,
                                           i * P:(i + 1) * P],
                                rhs,
                                start=(g == 0), stop=(g == n_g - 1),
                                perf_mode=mybir.MatmulPerfMode.DoubleRow)
                    for i in range(4):
                        ob = ps * 4 + i
                        ot = out_pool.tile([P, N_CH], f32, name="ot", tag="ot")
                        nc.scalar.activation(
                            ot[:], pss[i][:],
                            mybir.ActivationFunctionType.Copy,
                            bias=0.0, scale=float(scale))
                        nc.gpsimd.dma_start(
                            o_d[ob * P:(ob + 1) * P,
                                sc * N_CH:(sc + 1) * N_CH], ot[:])

    nc.compile()
    return nc


def _host_scale(shift_param):
    # 4x undoes the two 0.5 factors from binarizing to +/-0.5;
    # np.round is round-half-to-even, matching jnp.round.
    s = np.clip(np.float64(np.float32(shift_param)), -8.0, 0.0)
    return 4.0 * float(np.exp2(np.round(s)))


def _interleave_w_cols(w):
    """Host permutation so the device's grouped stationary layout pairs the
    same contraction rows as the packed moving layout: new col 256g+128j+p
    holds old col 256g+2p+j."""
    o, k = w.shape
    return np.ascontiguousarray(
        w.reshape(o, k // 256, 128, 2).transpose(0, 1, 3, 2).reshape(o, k))


def kernel(x, weight, threshold, shift_param):
    from concourse.bass_utils import run_bass_kernel_spmd

    scale = _host_scale(shift_param)
    nc = build_program(scale=scale)

    xf = np.ascontiguousarray(x.astype(np.float32).reshape(B * S, IN))
    wp = _interleave_w_cols(weight.astype(np.float32))
    in_maps = []
    for c in range(N_CORES):
        sl = slice(c * O_SHARD, (c + 1) * O_SHARD)
        in_maps.append({
            "x": xf,
            "w": np.ascontiguousarray(wp[sl]),
            "thr": np.ascontiguousarray(
                threshold[sl].astype(np.float32).reshape(O_SHARD, 1)),
        })

    res = run_bass_kernel_spmd(nc, in_maps, list(range(N_CORES)), trace=_TRACE)
    global _LAST_RESULTS
    _LAST_RESULTS = res
    shards = [res.results[c]["outT"] for c in range(N_CORES)]
    full_t = np.concatenate(shards, axis=0)  # [OUT, B*S]
    full = np.ascontiguousarray(full_t.T).reshape(B, S, OUT)
    return full.astype(np.float32)
